# revision 9
# baseline (speedup 1.0000x reference)
"""Trainium2 Bass kernel for nn_Pixel_spareformer (GNN message passing).

Self-contained: takes FULL inputs, shards across 8 NeuronCores internally,
returns the FULL [100000, 128] output.

Math (per branch, weights W_v/W_q [128,64], biases [64]):
    v = x @ Wv + bv
    q = LayerNorm64(x @ Wq + bq)
    s_e = (q[src_e] . q[dst_e]) / 64            # |s| <= 1 since ||q||^2 = 64
    out[n] = sum_{e: src=n} exp(s_e) v[dst_e] / sum_{e: src=n} exp(s_e)
(no max-subtraction needed: scores are bounded by Cauchy-Schwarz)
Then concat branches, BatchNorm (batch stats) + affine, leaky_relu(0.01).

Device strategy:
  - nodes sharded 12500/core (padded 12544); edges partitioned by src core
  - phase 1 (per core): q,v for LOCAL nodes -> local table [12544,128]
    ([q|v] interleaved rows) + local q table [12544,64]; AllGather the
    [q|v] tables to full [100352,128] per branch
  - phase 2: edges sorted by (src tile, dst window); per 128-node tile the
    segment softmax+sum is one PSUM-accumulated chain of mask matmuls:
       psum[n, 0:64] += maskT_chunk @ (exp(s) * v[dst]);  psum[n,64] += exp(s)
    gathers via the custom dma_gather (int16 idx -> 32768-row dst windows;
    src gather reads the local q table directly)
  - phase 3: per-channel sums via ones-matmul, AllReduce [1,256], normalize
    + leaky relu on node tiles.
"""
import math
import time

import numpy as np

import concourse.bass as bass
import concourse.tile as tile
from concourse import bacc, mybir
from concourse.masks import make_identity

# ---------------- problem constants (hardcoded per contract) ----------------
N = 100000
E = 1200000
DIN = 128
D = 64
NCORES = 8
NS = N // NCORES            # 12500 real nodes per core
TP = 98                     # 128-node tiles per core
NPAD = TP * 128             # 12544 padded nodes per core
NROWS = NCORES * NPAD       # 100352 table rows
WIN = 32768                 # dma_gather int16 window (table rows)
NWIN = (NROWS + WIN - 1) // WIN  # 4
NT = 3                      # phase-2 tiles per super-tile
GT = 2                      # phase-1 tiles per group
EPS = 1e-5
F32 = mybir.dt.float32
I16 = mybir.dt.int16
AX = mybir.AxisListType
OP = mybir.AluOpType
AF = mybir.ActivationFunctionType

# super-tile structure: [(tile0, ntiles), ...]
ST_LIST = [(t0, min(NT, TP - t0)) for t0 in range(0, TP, NT)]


# ---------------- host-side edge prep ----------------
def _wrap16(a):
    """int idx list [n] -> wrapped int16 [128, n/16] (16-wrap, replicated x8)."""
    n = len(a)
    w = a.astype(np.int16).reshape(n // 16, 16).T
    return np.tile(w, (8, 1))


def _prep_branch(src, dst):
    """Returns (S, per-core blobs dict)."""
    src = np.asarray(src, np.int64)
    dst = np.asarray(dst, np.int64)
    c_s = src // NS
    l_s = src % NS
    dstrow = (dst // NS) * NPAD + (dst % NS)
    tile_g = c_s * TP + l_s // 128          # global tile id
    shard = dstrow // WIN
    grp = tile_g * NWIN + shard
    order = np.argsort(grp, kind="stable")

    grp_s = grp[order]
    cnt = np.bincount(grp_s, minlength=NCORES * TP * NWIN)
    starts = np.zeros(NCORES * TP * NWIN, np.int64)
    starts[1:] = np.cumsum(cnt)[:-1]
    rank = np.arange(E, dtype=np.int64) - starts[grp_s]

    cntr = cnt.reshape(NCORES, TP, NWIN)
    S = np.maximum(1, (cntr.max(axis=(0, 1)) + 127) // 128).astype(np.int64)
    SC = int(S.sum())
    cumS = np.zeros(NWIN, np.int64)
    cumS[1:] = np.cumsum(S)[:-1]

    # chunk offset of each super-tile within the per-core chunk sequence
    st_of_tile = np.arange(TP) // NT
    nt_of_st = np.array([nt for _, nt in ST_LIST], np.int64)
    st_chunk_off = np.zeros(len(ST_LIST), np.int64)
    st_chunk_off[1:] = np.cumsum(nt_of_st * SC)[:-1]
    totch_core = int((nt_of_st * SC).sum())      # TP*SC
    totslots = totch_core * 128

    # per-edge slot position
    core_e = c_s[order]
    tl_e = (tile_g[order] % TP)
    st_e = st_of_tile[tl_e]
    tau_e = tl_e % NT
    sh_e = shard[order]
    offB = nt_of_st[st_e] * cumS[sh_e]
    pos = st_chunk_off[st_e] + offB + tau_e * S[sh_e] + rank // 128
    flat = pos * 128 + rank % 128

    dst_rel = (dstrow[order] - sh_e * WIN).astype(np.int16)
    qs_idx = l_s[order].astype(np.int16)
    sl_val = (l_s[order] % 128).astype(np.float32)

    dst_slots = np.zeros((NCORES, totslots), np.int16)
    qs_slots = np.zeros((NCORES, totslots), np.int16)
    sl_slots = np.full((NCORES, totslots), -1.0, np.float32)
    dst_slots[core_e, flat] = dst_rel
    qs_slots[core_e, flat] = qs_idx
    sl_slots[core_e, flat] = sl_val

    # build blobs
    qv_blob = np.empty((NCORES, 8 * totslots), np.int16)
    qs_blob = np.empty((NCORES, 8 * totslots), np.int16)
    sl_blob = np.empty((NCORES, totslots), np.float32)
    for c in range(NCORES):
        qv_parts, qs_parts, sl_parts = [], [], []
        for si, (t0, nt) in enumerate(ST_LIST):
            ch0 = st_chunk_off[si]
            ntsc = nt * SC
            # srcloc: [128, ntsc] row-major
            sl_parts.append(
                sl_slots[c, ch0 * 128:(ch0 + ntsc) * 128]
                .reshape(ntsc, 128).T.reshape(-1))
            qs_parts.append(
                _wrap16(qs_slots[c, ch0 * 128:(ch0 + ntsc) * 128]).reshape(-1))
            wparts = []
            for w in range(NWIN):
                a = ch0 + nt * cumS[w]
                b = a + nt * S[w]
                wparts.append(_wrap16(dst_slots[c, a * 128:b * 128]))
            qv_parts.append(np.concatenate(wparts, axis=1).reshape(-1))
        qv_blob[c] = np.concatenate(qv_parts)
        qs_blob[c] = np.concatenate(qs_parts)
        sl_blob[c] = np.concatenate(sl_parts)
    return tuple(int(x) for x in S), {
        "qv": qv_blob, "qs": qs_blob, "sl": sl_blob, "totslots": totslots}


# ---------------- device program ----------------
def _build_program(S_r, S_c, lens):
    nc = bacc.Bacc("TRN2", target_bir_lowering=False, debug=False,
                   num_devices=NCORES)
    xloc = nc.dram_tensor("xloc", [NPAD, DIN], F32, kind="ExternalInput")
    w4 = nc.dram_tensor("w4", [DIN, 4 * D], F32, kind="ExternalInput")
    b4 = nc.dram_tensor("b4", [1, 4 * D], F32, kind="ExternalInput")
    ga = nc.dram_tensor("ga", [1, DIN], F32, kind="ExternalInput")
    be = nc.dram_tensor("be", [1, DIN], F32, kind="ExternalInput")
    ins = {}
    for b in ("r", "c"):
        ins[b] = {
            "qv": nc.dram_tensor(f"qvidx_{b}", [lens[b]["qv"]], I16,
                                 kind="ExternalInput"),
            "qs": nc.dram_tensor(f"qsidx_{b}", [lens[b]["qs"]], I16,
                                 kind="ExternalInput"),
            "sl": nc.dram_tensor(f"sl_{b}", [lens[b]["sl"]], F32,
                                 kind="ExternalInput"),
        }
    outp = nc.dram_tensor("outp", [NPAD, DIN], F32, kind="ExternalOutput")

    tabl = {b: nc.dram_tensor(f"tabl_{b}", [NPAD, DIN], F32) for b in "rc"}
    tab = {b: nc.dram_tensor(f"tab_{b}", [NROWS, DIN], F32,
                             addr_space="Shared") for b in "rc"}
    qloc = {b: nc.dram_tensor(f"qloc_{b}", [NPAD, D], F32) for b in "rc"}
    pre = nc.dram_tensor("pre", [NPAD, DIN], F32)
    ccin = nc.dram_tensor("ccin", [1, 2 * DIN], F32)
    ccout = nc.dram_tensor("ccout", [1, 2 * DIN], F32, addr_space="Shared")

    S = {"r": S_r, "c": S_c}
    SC = {b: sum(S[b]) for b in "rc"}
    cumS = {b: [sum(S[b][:w]) for w in range(NWIN)] for b in "rc"}

    with tile.TileContext(nc) as tc, \
            tc.tile_pool(name="const", bufs=1) as cp:
        with (
            tc.tile_pool(name="p1", bufs=3) as p1,
            tc.tile_pool(name="ps_a", bufs=2, space="PSUM") as psa,
            tc.tile_pool(name="ps_b", bufs=2, space="PSUM") as psb,
        ):
            # constants
            ident = cp.tile([128, 128], F32)
            make_identity(nc, ident[:])
            iota_i = cp.tile([128, 128], mybir.dt.int32)
            nc.gpsimd.iota(iota_i[:], pattern=[[1, 128]], base=0,
                           channel_multiplier=0)
            iota_f = cp.tile([128, 128], F32)
            nc.vector.tensor_copy(iota_f[:], iota_i[:])
            ones1 = cp.tile([1, 128], F32)
            nc.vector.memset(ones1[:], 1.0)
            ones128 = cp.tile([128, 1], F32)
            nc.vector.memset(ones128[:], 1.0)
            w4_sb = cp.tile([128, 4 * D], F32)
            nc.sync.dma_start(w4_sb[:], w4[:])
            b4_sb = cp.tile([1, 4 * D], F32)
            nc.sync.dma_start(b4_sb[:], b4[:])
            ga_sb = cp.tile([1, DIN], F32)
            nc.sync.dma_start(ga_sb[:], ga[:])
            be_sb = cp.tile([1, DIN], F32)
            nc.sync.dma_start(be_sb[:], be[:])
            epst = cp.tile([128, 1], F32)
            nc.vector.memset(epst[:], EPS)

            # ---------- phase 1: local q,v ----------
            for g in range(TP // GT):
                r0, r1 = g * GT * 128, (g + 1) * GT * 128
                xt = p1.tile([128, GT, 128], F32, tag="xt")
                nc.sync.dma_start(
                    xt[:], xloc[r0:r1, :].rearrange("(g p) d -> p g d", p=128))
                tps = psa.tile([128, GT * 128], F32, tag="tps")
                for i in range(GT):
                    nc.tensor.transpose(
                        tps[:, i * 128:(i + 1) * 128], xt[:, i, :], ident[:])
                xT = p1.tile([128, GT * 128], F32, tag="xT")
                nc.vector.tensor_copy(xT[:], tps[:])
                qps = psb.tile([128, GT, 4 * D], F32, tag="qps")
                for i in range(GT):
                    nc.tensor.matmul(qps[:, i, :],
                                     lhsT=xT[:, i * 128:(i + 1) * 128],
                                     rhs=w4_sb[:], start=True, stop=False)
                    nc.tensor.matmul(qps[:, i, :], lhsT=ones1[:],
                                     rhs=b4_sb[:], start=False, stop=True)
                for bi, b in enumerate("rc"):
                    q_ps = qps[:, :, 2 * bi * D:(2 * bi + 1) * D]
                    v_ps = qps[:, :, (2 * bi + 1) * D:(2 * bi + 2) * D]
                    m = p1.tile([128, GT], F32, tag="m")
                    nc.vector.tensor_reduce(m[:], q_ps, axis=AX.X, op=OP.add)
                    m2 = p1.tile([128, GT], F32, tag="m2")
                    nc.vector.tensor_scalar_mul(m2[:], m[:], 1.0 / D)
                    qc = p1.tile([128, GT, D], F32, tag="qc")
                    nc.vector.tensor_tensor(
                        qc[:], q_ps,
                        m2[:].rearrange("p g -> p g ()").to_broadcast(
                            [128, GT, D]),
                        op=OP.subtract)
                    sq = p1.tile([128, GT, D], F32, tag="sq")
                    nc.vector.tensor_tensor(sq[:], qc[:], qc[:], op=OP.mult)
                    va = p1.tile([128, GT], F32, tag="va")
                    nc.vector.tensor_reduce(va[:], sq[:], axis=AX.X, op=OP.add)
                    sd = p1.tile([128, GT], F32, tag="sd")
                    nc.scalar.activation(sd[:], va[:], AF.Sqrt,
                                         bias=epst[:, 0:1], scale=1.0 / D)
                    rs = p1.tile([128, GT], F32, tag="rs")
                    nc.vector.reciprocal(rs[:], sd[:])
                    qvt = p1.tile([128, GT, 128], F32, tag="qvt")
                    nc.vector.tensor_tensor(
                        qvt[:, :, 0:D], qc[:],
                        rs[:].rearrange("p g -> p g ()").to_broadcast(
                            [128, GT, D]),
                        op=OP.mult)
                    nc.vector.tensor_copy(qvt[:, :, D:128], v_ps)
                    nc.scalar.dma_start(
                        tabl[b][r0:r1, :].rearrange("(g p) d -> p g d", p=128),
                        qvt[:])
                    nc.scalar.dma_start(
                        qloc[b][r0:r1, :].rearrange("(g p) d -> p g d", p=128),
                        qvt[:, :, 0:D])

            for b in "rc":
                nc.gpsimd.collective_compute(
                    "AllGather", OP.bypass,
                    replica_groups=[list(range(NCORES))],
                    ins=[tabl[b][:]], outs=[tab[b][:]])

        # ---------- phase 2 ----------
        with (
            tc.tile_pool(name="gbuf", bufs=2) as gb,
            tc.tile_pool(name="work", bufs=2) as wk,
            tc.tile_pool(name="small", bufs=4) as sm,
            tc.tile_pool(name="ps_acc", bufs=4, space="PSUM") as psc,
        ):
            for b in "rc":
                scb = SC[b]
                qv_off = 0   # int16 elements consumed from qv blob
                qs_off = 0
                sl_off = 0
                for (t0, nt) in ST_LIST:
                    totch = nt * scb
                    idxv = sm.tile([128, totch * 8], I16, tag="idxv")
                    nc.sync.dma_start(
                        idxv[:],
                        ins[b]["qv"][qv_off:qv_off + 128 * totch * 8]
                        .rearrange("(p s) -> p s", p=128))
                    idxs = sm.tile([128, totch * 8], I16, tag="idxs")
                    nc.sync.dma_start(
                        idxs[:],
                        ins[b]["qs"][qs_off:qs_off + 128 * totch * 8]
                        .rearrange("(p s) -> p s", p=128))
                    slt = sm.tile([128, totch], F32, tag="slt")
                    nc.sync.dma_start(
                        slt[:],
                        ins[b]["sl"][sl_off:sl_off + 128 * totch]
                        .rearrange("(p s) -> p s", p=128))
                    qv_off += 128 * totch * 8
                    qs_off += 128 * totch * 8
                    sl_off += 128 * totch

                    qvb = gb.tile([128, totch, 128], F32, tag="qvb")
                    wo8 = 0
                    for w in range(NWIN):
                        ch_a = nt * cumS[b][w]
                        n_ch = nt * S[b][w]
                        nc.gpsimd.dma_gather(
                            out_ap=qvb[:, ch_a:ch_a + n_ch, :],
                            in_ap=tab[b][w * WIN:, :],
                            idxs_ap=idxv[:, wo8:wo8 + n_ch * 8],
                            num_idxs=n_ch * 128,
                            num_idxs_reg=n_ch * 128,
                            elem_size=128, single_packet=False)
                        wo8 += n_ch * 8
                    qsb = gb.tile([128, totch, D], F32, tag="qsb")
                    nc.gpsimd.dma_gather(
                        out_ap=qsb[:], in_ap=qloc[b][:],
                        idxs_ap=idxs[:],
                        num_idxs=totch * 128, num_idxs_reg=totch * 128,
                        elem_size=D, single_packet=False)

                    # scores -> exp -> weighted values
                    nc.vector.tensor_tensor(qsb[:], qsb[:], qvb[:, :, 0:D],
                                            op=OP.mult)
                    sc_t = sm.tile([128, totch], F32, tag="sc")
                    nc.vector.tensor_reduce(sc_t[:], qsb[:], axis=AX.X,
                                            op=OP.add)
                    e_t = sm.tile([128, totch], F32, tag="e")
                    nc.scalar.activation(e_t[:], sc_t[:], AF.Exp,
                                         scale=1.0 / D)
                    agg = wk.tile([128, totch, D + 1], F32, tag="agg")
                    nc.any.tensor_tensor(
                        agg[:, :, 0:D], qvb[:, :, D:128],
                        e_t[:].rearrange("p c -> p c ()").to_broadcast(
                            [128, totch, D]),
                        op=OP.mult)
                    nc.vector.tensor_copy(
                        agg[:, :, D:D + 1],
                        e_t[:].rearrange("p c -> p c ()"))
                    mask = wk.tile([128, totch, 128], F32, tag="mask")
                    nc.any.tensor_tensor(
                        mask[:],
                        iota_f[:].rearrange("p d -> p () d").to_broadcast(
                            [128, totch, 128]),
                        slt[:].rearrange("p c -> p c ()").to_broadcast(
                            [128, totch, 128]),
                        op=OP.is_equal)

                    for tl in range(nt):
                        ps = psc.tile([128, D + 1], F32, tag="ps")
                        poss = [nt * cumS[b][w] + tl * S[b][w] + k
                                for w in range(NWIN) for k in range(S[b][w])]
                        for i, cp_ in enumerate(poss):
                            nc.tensor.matmul(
                                ps[:], lhsT=mask[:, cp_, :],
                                rhs=agg[:, cp_, :],
                                start=(i == 0), stop=(i == len(poss) - 1))
                        den = sm.tile([128, 1], F32, tag="den")
                        nc.vector.tensor_scalar_max(den[:], ps[:, D:D + 1],
                                                    1e-30)
                        rd = sm.tile([128, 1], F32, tag="rd")
                        nc.vector.reciprocal(rd[:], den[:])
                        ot = sm.tile([128, D], F32, tag="ot")
                        nc.vector.tensor_tensor(
                            ot[:], ps[:, 0:D], rd[:].to_broadcast([128, D]),
                            op=OP.mult)
                        t = t0 + tl
                        bc = 0 if b == "r" else D
                        nc.scalar.dma_start(
                            pre[t * 128:(t + 1) * 128, bc:bc + D], ot[:])

        # ---------- phase 3: batchnorm + leaky relu ----------
        with (
            tc.tile_pool(name="p3", bufs=3) as p3,
            tc.tile_pool(name="p3s", bufs=1) as p3s,
            tc.tile_pool(name="ps_st", bufs=1, space="PSUM") as pst,
        ):
            sps = pst.tile([1, 2 * DIN], F32, tag="sps")
            for t in range(TP):
                pt = p3.tile([128, DIN], F32, tag="pt")
                nc.sync.dma_start(pt[:], pre[t * 128:(t + 1) * 128, :])
                sq3 = p3.tile([128, DIN], F32, tag="sq3")
                nc.scalar.activation(sq3[:], pt[:], AF.Square)
                nc.tensor.matmul(sps[:, 0:DIN], lhsT=ones128[:], rhs=pt[:],
                                 start=(t == 0), stop=(t == TP - 1))
                nc.tensor.matmul(sps[:, DIN:2 * DIN], lhsT=ones128[:],
                                 rhs=sq3[:], start=(t == 0),
                                 stop=(t == TP - 1))
            st_sb = p3s.tile([1, 2 * DIN], F32, tag="stsb")
            nc.vector.tensor_copy(st_sb[:], sps[:])
            nc.sync.dma_start(ccin[:], st_sb[:])
            nc.gpsimd.collective_compute(
                "AllReduce", OP.add, replica_groups=[list(range(NCORES))],
                ins=[ccin[:]], outs=[ccout[:]])
            cc_sb = p3s.tile([1, 2 * DIN], F32, tag="ccsb")
            nc.sync.dma_start(cc_sb[:], ccout[:])
            mu = p3s.tile([1, DIN], F32, tag="mu")
            nc.vector.tensor_scalar_mul(mu[:], cc_sb[:, 0:DIN], 1.0 / N)
            ex2 = p3s.tile([1, DIN], F32, tag="ex2")
            nc.vector.tensor_scalar_mul(ex2[:], cc_sb[:, DIN:2 * DIN], 1.0 / N)
            msq = p3s.tile([1, DIN], F32, tag="msq")
            nc.vector.tensor_tensor(msq[:], mu[:], mu[:], op=OP.mult)
            var = p3s.tile([1, DIN], F32, tag="var")
            nc.vector.tensor_tensor(var[:], ex2[:], msq[:], op=OP.subtract)
            sd3 = p3s.tile([1, DIN], F32, tag="sd3")
            nc.scalar.activation(sd3[:], var[:], AF.Sqrt, bias=epst[0:1, 0:1])
            rs3 = p3s.tile([1, DIN], F32, tag="rs3")
            nc.vector.reciprocal(rs3[:], sd3[:])
            A = p3s.tile([1, DIN], F32, tag="A")
            nc.vector.tensor_tensor(A[:], rs3[:], ga_sb[:], op=OP.mult)
            muA = p3s.tile([1, DIN], F32, tag="muA")
            nc.vector.tensor_tensor(muA[:], mu[:], A[:], op=OP.mult)
            B = p3s.tile([1, DIN], F32, tag="B")
            nc.vector.tensor_tensor(B[:], be_sb[:], muA[:], op=OP.subtract)
            abps = pst.tile([128, 2 * DIN], F32, tag="abps")
            nc.tensor.matmul(abps[:, 0:DIN], lhsT=ones1[:], rhs=A[:],
                             start=True, stop=True)
            nc.tensor.matmul(abps[:, DIN:2 * DIN], lhsT=ones1[:], rhs=B[:],
                             start=True, stop=True)
            ab = p3s.tile([128, 2 * DIN], F32, tag="ab")
            nc.vector.tensor_copy(ab[:], abps[:])
            for t in range(TP):
                pt = p3.tile([128, DIN], F32, tag="pt2")
                nc.sync.dma_start(pt[:], pre[t * 128:(t + 1) * 128, :])
                y = p3.tile([128, DIN], F32, tag="y")
                nc.vector.tensor_tensor(y[:], pt[:], ab[:, 0:DIN], op=OP.mult)
                nc.vector.tensor_tensor(y[:], y[:], ab[:, DIN:2 * DIN],
                                        op=OP.add)
                ys = p3.tile([128, DIN], F32, tag="ys")
                nc.vector.tensor_scalar_mul(ys[:], y[:], 0.01)
                o = p3.tile([128, DIN], F32, tag="o")
                nc.vector.tensor_tensor(o[:], y[:], ys[:], op=OP.max)
                nc.scalar.dma_start(outp[t * 128:(t + 1) * 128, :], o[:])

    nc.compile()
    return nc


# ---------------- cached executable ----------------
_CACHE = {}


class _Exe:
    def __init__(self, nc):
        import jax
        from jax.experimental.shard_map import shard_map
        from jax.sharding import Mesh, PartitionSpec
        from concourse import bass2jax

        bass2jax.install_neuronx_cc_hook()
        self.jax = jax
        partition_name = (nc.partition_id_tensor.name
                          if nc.partition_id_tensor else None)
        in_names, out_names, out_avals, zero_shapes = [], [], [], []
        for alloc in nc.m.functions[0].allocations:
            if not isinstance(alloc, mybir.MemoryLocationSet):
                continue
            name = alloc.memorylocations[0].name
            if alloc.kind == "ExternalInput":
                if name != partition_name:
                    in_names.append(name)
            elif alloc.kind == "ExternalOutput":
                out_names.append(name)
                shape = tuple(alloc.tensor_shape)
                dtype = mybir.dt.np(alloc.dtype)
                out_avals.append(jax.core.ShapedArray(shape, dtype))
                zero_shapes.append((shape, dtype))
        self.in_names = in_names
        self.out_names = out_names
        self.zero_shapes = zero_shapes
        n_params = len(in_names)
        n_outs = len(out_names)
        all_names = list(in_names) + list(out_names)
        if partition_name is not None:
            all_names.append(partition_name)

        def _body(*args):
            operands = list(args)
            if partition_name is not None:
                operands.append(bass2jax.partition_id_tensor())
            outs = bass2jax._bass_exec_p.bind(
                *operands,
                out_avals=tuple(out_avals),
                in_names=tuple(all_names),
                out_names=tuple(out_names),
                lowering_input_output_aliases=(),
                sim_require_finite=True,
                sim_require_nnan=True,
                nc=nc,
            )
            return tuple(outs)

        devices = jax.devices()[:NCORES]
        mesh = Mesh(np.asarray(devices), ("core",))
        in_specs = (PartitionSpec("core"),) * (n_params + n_outs)
        out_specs = (PartitionSpec("core"),) * n_outs
        self.sharded = jax.jit(
            shard_map(_body, mesh=mesh, in_specs=in_specs,
                      out_specs=out_specs, check_rep=False),
            donate_argnums=tuple(range(n_params, n_params + n_outs)),
            keep_unused=True)

    def make_zeros(self):
        return [np.zeros((NCORES * s[0], *s[1:]), d)
                for s, d in self.zero_shapes]

    def run(self, concat_in, zeros):
        out = self.sharded(*concat_in, *zeros)
        self.jax.block_until_ready(out)
        return out


LAST_EXEC_SECONDS = None


def _prepare(inputs):
    """Host prep: returns (key, concat_in builder data)."""
    x = np.asarray(inputs["x"], np.float32)
    xp = np.zeros((NROWS, DIN), np.float32)
    for c in range(NCORES):
        xp[c * NPAD:c * NPAD + NS] = x[c * NS:(c + 1) * NS]

    S_r, br = _prep_branch(inputs["rowsrc"], inputs["rowdst"])
    S_c, bc = _prep_branch(inputs["colsrc"], inputs["coldst"])

    w4 = np.concatenate([
        np.asarray(inputs["Wrq"], np.float32),
        np.asarray(inputs["Wrv"], np.float32),
        np.asarray(inputs["Wcq"], np.float32),
        np.asarray(inputs["Wcv"], np.float32)], axis=1)
    b4 = np.concatenate([
        np.asarray(inputs["brq"], np.float32),
        np.asarray(inputs["brv"], np.float32),
        np.asarray(inputs["bcq"], np.float32),
        np.asarray(inputs["bcv"], np.float32)])[None, :]

    lens = {bn: {"qv": blob["qv"].shape[1], "qs": blob["qs"].shape[1],
                 "sl": blob["sl"].shape[1]}
            for bn, blob in (("r", br), ("c", bc))}
    key = (S_r, S_c, lens["r"]["qv"], lens["c"]["qv"])

    concat_in = {
        "xloc": xp,
        "w4": np.tile(w4, (NCORES, 1)),
        "b4": np.tile(b4, (NCORES, 1)),
        "ga": np.tile(np.asarray(inputs["gamma"], np.float32)[None, :],
                      (NCORES, 1)),
        "be": np.tile(np.asarray(inputs["beta"], np.float32)[None, :],
                      (NCORES, 1)),
        "qvidx_r": br["qv"].reshape(-1),
        "qsidx_r": br["qs"].reshape(-1),
        "sl_r": br["sl"].reshape(-1),
        "qvidx_c": bc["qv"].reshape(-1),
        "qsidx_c": bc["qs"].reshape(-1),
        "sl_c": bc["sl"].reshape(-1),
    }
    return key, (S_r, S_c, lens), concat_in


def kernel(**inputs) -> np.ndarray:
    global LAST_EXEC_SECONDS
    key, build_args, concat_in = _prepare(inputs)
    if key not in _CACHE:
        nc = _build_program(build_args[0], build_args[1], build_args[2])
        _CACHE[key] = _Exe(nc)
    exe = _CACHE[key]

    import jax
    dev_in = [jax.device_put(concat_in[nm]) for nm in exe.in_names]
    zeros = [jax.device_put(z) for z in exe.make_zeros()]
    jax.block_until_ready(dev_in)
    jax.block_until_ready(zeros)
    t0 = time.perf_counter()
    outs = exe.run(dev_in, zeros)
    LAST_EXEC_SECONDS = time.perf_counter() - t0

    outp = np.asarray(outs[exe.out_names.index("outp")])
    outp = outp.reshape(NCORES, NPAD, DIN)
    full = np.empty((N, DIN), np.float32)
    for c in range(NCORES):
        full[c * NS:(c + 1) * NS] = outp[c, :NS]
    return full
